# revision 27
# baseline (speedup 1.0000x reference)
"""Causal multi-head attention (RoPE) on 8 TRN2 NeuronCores.

Sharding: Megatron-style head parallelism. Each core owns 2 of the 16 heads:
it computes q/k/v projections for its 128 output features (2 heads x 64),
applies interleaved-pair RoPE (rotation done as a PE matmul with a constant
pair-swap matrix R, tables precomputed host-side), runs causal attention for
its (batch, head) pairs in the transposed orientation S^T = K^T Q so that no
on-chip transposes of the attention matrix are needed, and accumulates
attn^T-weighted V with an extra all-ones V column that yields the softmax
denominators for free. An AllToAll then redistributes the per-head outputs
from head-sharded to token-sharded layout, and each core computes the final
output projection for its token slice. A tiny dummy AllToAll is issued at
kernel start to absorb the one-time collective warmup (~40us) while the
engines compute.

Schedule: qkv(tile t) and attention(qt t) are interleaved per 512-token
block so attention starts as soon as the first tile's k/v are ready, hiding
the input DMA behind compute. Sub-collectives fire per 1024 tokens (the last
1024 split into 2x512 so the tail exposes only a half-size AllToAll).

Causal masking: only the mixed 128x128 block per diagonal chunk is masked
(one small DVE multiply); fully-masked columns are simply skipped by
column-slicing the exp and the AV matmul. Softmax normalization runs off the
PE critical path: reciprocal_approx_fast (DVE) -> partition_broadcast (Pool)
-> one DVE multiply.

Compute dtype: bf16 operands with fp32 PSUM accumulation. (float32r would
be more precise, but f32r matmuls do not register as PE activity for the HAM
clock gate, so the PE stays throttled at 1.2 GHz; bf16 runs at 2.4 GHz.)
"""

import sys

sys.path.insert(0, "/opt/trn_rl_repo")

import numpy as np

B, L, D, N, H = 2, 2048, 1024, 16, 64
T = B * L            # 4096 tokens, batch-major
NC = 8               # cores
HPC = N // NC        # 2 heads per core
W = HPC * H          # 128 projection features per core
THETA = 10000.0
VBLK = 256           # v_sb block stride: [h0 64 | ones 64 | h1 64 | ones 64]

# sub-collectives: (batch, qts, tokens-per-core)
SUBS = [(0, (0, 1), 128), (0, (2, 3), 128), (1, (0, 1), 128),
        (1, (2, 3), 128)]
SUB_ROW = [0, 128, 256, 384]     # out_ext row offset per sub
SUB_OG = [0, 1024, 2048, 3072]   # og_all col offset per sub

_CACHED = {}


def _build():
    import concourse.bass as bass
    import concourse.mybir as mybir
    import concourse.tile as tile

    F32 = mybir.dt.float32
    BF16 = mybir.dt.bfloat16
    AF = mybir.ActivationFunctionType

    # ---- fix: stock _drain_and_barrier overflows the 2-slot sync encoding
    import re as _re
    from concourse.vector_clock import ScopedClock, VectorClock

    def _split_drain_and_barrier(self, tick_clock, wait_clock):
        gc = tick_clock.global_clock
        ticks = [int(v) for v in _re.findall(r"-?\d+", str(gc))]
        for proc, t in enumerate(ticks):
            if t <= 0:
                continue
            sub = VectorClock()
            sub.require_at_least(proc, t)
            d = self.nc.sync.drain()
            wait_clock.add_sem_waits(d.ins, ScopedClock({None: sub}))
        self.nc.all_engine_barrier()
        assert self.sems is not None
        popped = self.nc._tile_sem_poison_stack.pop()
        assert popped is self._sem_poison
        self.nc.clear_and_free_semaphores(list(self.sems.allocated().values()))
        self.nc.all_engine_barrier()

    tile.TileContext._drain_and_barrier = _split_drain_and_barrier

    nc = bass.Bass()

    xT_ext = nc.declare_dram_parameter("xT", [T // 512, 128, (D // 128) * 512], BF16, isOutput=False)
    wq_ext = nc.declare_dram_parameter("wq", [128, D], BF16, isOutput=False)
    wk_ext = nc.declare_dram_parameter("wk", [128, D], BF16, isOutput=False)
    wv_ext = nc.declare_dram_parameter("wv", [128, D], BF16, isOutput=False)
    bq_ext = nc.declare_dram_parameter("bq", [W, 1], F32, isOutput=False)
    bk_ext = nc.declare_dram_parameter("bk", [W, 1], F32, isOutput=False)
    bv_ext = nc.declare_dram_parameter("bv", [W, 1], F32, isOutput=False)
    cos_ext = nc.declare_dram_parameter("cosT", [W, L], BF16, isOutput=False)
    sin_ext = nc.declare_dram_parameter("sinT", [W, L], BF16, isOutput=False)
    rmat_ext = nc.declare_dram_parameter("rmat", [128, 128], BF16, isOutput=False)
    ident_ext = nc.declare_dram_parameter("ident", [128, 128], BF16, isOutput=False)
    mixu_ext = nc.declare_dram_parameter("mixu", [128, 128], BF16, isOutput=False)
    ones_ext = nc.declare_dram_parameter("onesc", [128, 128], BF16, isOutput=False)
    wo_ext = nc.declare_dram_parameter("woT", [128, NC * D], BF16, isOutput=False)
    bo_ext = nc.declare_dram_parameter("bo", [1, D], BF16, isOutput=False)
    out_ext = nc.declare_dram_parameter("out", [T // NC, D], BF16, isOutput=True)

    TT = T // 512      # 8 token tiles of 512
    KD = D // 128      # 8 contraction chunks

    with tile.TileContext(nc) as tc, nc.allow_low_precision(reason="bf16 attn"):
        with tc.tile_pool(name="dram", bufs=1, space="DRAM") as dram:
            # dummy collective: absorbs one-time collective warmup while
            # compute runs
            dum_in = dram.tile([NC, 128, 128], BF16)
            dum_out = dram.tile([NC, 128, 128], BF16)
            nc.gpsimd.collective_compute(
                "AllToAll", mybir.AluOpType.bypass,
                replica_groups=[list(range(NC))],
                ins=[dum_in[:].opt()], outs=[dum_out[:].opt()],
            )
            cins = [dram.tile([NC, 128, n], BF16, name=f"cin{i}")
                    for i, (_, _, n) in enumerate(SUBS)]
            couts = [dram.tile([NC, 128, n], BF16, name=f"cout{i}")
                     for i, (_, _, n) in enumerate(SUBS)]

            cpool = tc.alloc_tile_pool(name="const", bufs=1)
            work = tc.alloc_tile_pool(name="work", bufs=1)
            xtpool = tc.alloc_tile_pool(name="xt", bufs=5)
            t1pool = tc.alloc_tile_pool(name="p1t", bufs=3)
            atpool = tc.alloc_tile_pool(name="att", bufs=6)
            nrmpool = tc.alloc_tile_pool(name="nrm", bufs=2)
            obpool = tc.alloc_tile_pool(name="ob", bufs=2)
            psA = tc.alloc_tile_pool(name="psA", bufs=2, space="PSUM")
            psT = tc.alloc_tile_pool(name="psT", bufs=2, space="PSUM")
            pso = tc.alloc_tile_pool(name="pso", bufs=2, space="PSUM")

            # ---- constants (spread across DMA queues for startup bandwidth)
            wq_sb = cpool.tile([128, KD * 128], BF16)
            nc.scalar.dma_start(wq_sb[:], wq_ext[:])
            bq_sb = cpool.tile([W, 1], F32)
            nc.scalar.dma_start(bq_sb[:], bq_ext[:])
            wk_sb = cpool.tile([128, KD * 128], BF16)
            nc.scalar.dma_start(wk_sb[:], wk_ext[:])
            wv_sb = cpool.tile([128, KD * 128], BF16)
            nc.scalar.dma_start(wv_sb[:], wv_ext[:])
            bk_sb = cpool.tile([W, 1], F32)
            nc.scalar.dma_start(bk_sb[:], bk_ext[:])
            bv_sb = cpool.tile([W, 1], F32)
            nc.scalar.dma_start(bv_sb[:], bv_ext[:])
            rmat_sb = cpool.tile([128, 128], BF16)
            nc.scalar.dma_start(rmat_sb[:], rmat_ext[:])
            ident_sb = cpool.tile([128, 128], BF16)
            nc.scalar.dma_start(ident_sb[:], ident_ext[:])
            mixu_sb = cpool.tile([128, 128], BF16)
            nc.scalar.dma_start(mixu_sb[:], mixu_ext[:])
            # cos/sin loaded per-512-column slice inside emit_qkv so they
            # never delay the first x tile
            cos_sb = cpool.tile([W, L], BF16)
            sin_sb = cpool.tile([W, L], BF16)
            ones_sb = cpool.tile([128, 128], BF16)
            nc.gpsimd.dma_start(ones_sb[:], ones_ext[:])
            wo_sb = cpool.tile([128, NC * D], BF16)
            bo_sb = cpool.tile([1, D], BF16)

            # ---- long-lived working tensors
            q_ro = work.tile([128, T], BF16)
            k_ro = work.tile([128, T], BF16)
            v_sb = work.tile([128, 32 * VBLK], BF16)
            o_sb = work.tile([128, T], BF16)
            og_all = work.tile([128, T], BF16)
            # ones blocks of v_sb: the AV stationary is [v(64) | ones(64)]
            # per head so PSUM rows 64-127 come out holding 64 copies of the
            # softmax denominator (no separate broadcast needed)
            nc.vector.memset(v_sb[:], 1.0)

            def emit_qkv(tt):
                xt = xtpool.tile([128, KD * 512], BF16, name="xt", tag="xt")
                if tt == 0:
                    for quarter in range(4):
                        c0 = quarter * KD * 128
                        nc.sync.dma_start(xt[:, c0:c0 + KD * 128],
                                          xT_ext[tt][:, c0:c0 + KD * 128])
                elif tt % 2 == 0:
                    nc.sync.dma_start(xt[:], xT_ext[tt])
                else:
                    nc.gpsimd.dma_start(xt[:], xT_ext[tt])
                xts = [xt[:, k * 512:(k + 1) * 512] for k in range(KD)]
                lcol = (tt % (TT // B)) * 512
                if tt < TT // B:
                    nc.sync.dma_start(cos_sb[:, lcol:lcol + 512],
                                      cos_ext[:, lcol:lcol + 512])
                    nc.sync.dma_start(sin_sb[:, lcol:lcol + 512],
                                      sin_ext[:, lcol:lcol + 512])

                for wsb, bsb, dst in ((wq_sb, bq_sb, q_ro),
                                      (wk_sb, bk_sb, k_ro)):
                    ps = psA.tile([128, 512], F32, tag="pj", name="pj")
                    for k in range(KD):
                        nc.tensor.matmul(ps[:], wsb[:, k * 128:(k + 1) * 128],
                                         xts[k], start=(k == 0),
                                         stop=(k == KD - 1))
                    bs = t1pool.tile([128, 512], BF16, tag="bs", name="bs")
                    nc.scalar.activation(bs[:], ps[:], AF.Identity, bias=bsb[:])
                    rot = psA.tile([128, 512], F32, tag="pj", name="rot")
                    nc.tensor.matmul(rot[:], rmat_sb[:], bs[:],
                                     start=True, stop=True)
                    t1 = t1pool.tile([128, 512], BF16, tag="t1", name="t1")
                    nc.vector.tensor_mul(t1[:], bs[:], cos_sb[:, lcol:lcol + 512])
                    t2 = t1pool.tile([128, 512], BF16, tag="t2", name="t2")
                    nc.vector.tensor_mul(t2[:], rot[:], sin_sb[:, lcol:lcol + 512])
                    nc.vector.tensor_add(dst[:, tt * 512:(tt + 1) * 512],
                                         t1[:], t2[:])

                ps = psA.tile([128, 512], F32, tag="pj", name="pv")
                for k in range(KD):
                    nc.tensor.matmul(ps[:], wv_sb[:, k * 128:(k + 1) * 128],
                                     xts[k], start=(k == 0), stop=(k == KD - 1))
                vbs = t1pool.tile([128, 512], BF16, tag="bs", name="vbs")
                nc.scalar.activation(vbs[:], ps[:], AF.Identity, bias=bv_sb[:])
                for s in range(4):
                    vt = psA.tile([128, 128], BF16, tag="pj", name="vt")
                    nc.tensor.transpose(vt[:], vbs[:, s * 128:(s + 1) * 128],
                                        ident_sb[:])
                    blk = (tt * 4 + s) * VBLK
                    nc.vector.tensor_copy(v_sb[:, blk:blk + 64], vt[:, 0:64])
                    nc.vector.tensor_copy(v_sb[:, blk + 128:blk + 192],
                                          vt[:, 64:128])

            def emit_attention(b, qt):
                tof = b * L
                vb = b * (L // 128)
                nkc = 4 * qt + 4
                opss = [pso.tile([128, 512], F32, tag="ops", name="ops")
                        for _ in range(HPC)]
                ats = {}

                def emit_sT(kc):
                    r = kc - 4 * qt
                    cs = 128 * r if r > 0 else 0    # skip fully-masked cols
                    sT2 = psT.tile([128, 1024], F32, tag="sT", name="sT")
                    for hl in range(HPC):
                        nc.tensor.matmul(
                            sT2[:, hl * 512 + cs:(hl + 1) * 512],
                            k_ro[64 * hl:64 * hl + 64,
                                 tof + kc * 128:tof + kc * 128 + 128],
                            q_ro[64 * hl:64 * hl + 64,
                                 tof + qt * 512 + cs:tof + qt * 512 + 512],
                            start=True, stop=True)
                    at2 = atpool.tile([128, 1024], BF16, tag="at", name="at")
                    if r <= 0:
                        nc.scalar.activation(at2[:], sT2[:], AF.Exp)
                    else:
                        c0 = 128 * r
                        for hl in range(HPC):
                            nc.scalar.activation(
                                at2[:, hl * 512 + c0:(hl + 1) * 512],
                                sT2[:, hl * 512 + c0:(hl + 1) * 512], AF.Exp)
                    if r >= 0:
                        c0 = 128 * r
                        for hl in range(HPC):
                            nc.vector.tensor_mul(
                                at2[:, hl * 512 + c0:hl * 512 + c0 + 128],
                                at2[:, hl * 512 + c0:hl * 512 + c0 + 128],
                                mixu_sb[:])
                    ats[kc] = at2

                def emit_av(kc, first, last):
                    r = kc - 4 * qt
                    c0 = 128 * r if r > 0 else 0
                    for hl in range(HPC):
                        nc.tensor.matmul(
                            opss[hl][:, c0:512],
                            v_sb[:, (vb + kc) * VBLK + 128 * hl:
                                 (vb + kc) * VBLK + 128 * hl + 128],
                            ats[kc][:, hl * 512 + c0:(hl + 1) * 512],
                            start=(kc == first), stop=(kc == last),
                            skip_group_check=True)

                # diagonal (masked) chunks first
                kcs = list(range(4 * qt, nkc)) + list(range(0, 4 * qt))
                first, last = kcs[0], kcs[-1]
                emit_sT(first)
                for i in range(1, nkc):
                    emit_sT(kcs[i])
                    emit_av(kcs[i - 1], first, last)
                emit_av(last, first, last)

                for hl in range(HPC):
                    hof = 64 * hl
                    ops = opss[hl]
                    # 1/d = exp(-ln d); rows 64-127 hold 64 denominator copies
                    lnt = nrmpool.tile([64, 512], F32, tag="lnt", name="lnt")
                    nc.scalar.activation(lnt[:], ops[64:128, :], AF.Ln)
                    bcs = nrmpool.tile([64, 512], F32, tag="bcs", name="bcs")
                    nc.scalar.activation(bcs[:], lnt[:], AF.Exp, scale=-1.0)
                    nc.vector.tensor_mul(
                        o_sb[hof:hof + 64,
                             tof + qt * 512:tof + qt * 512 + 512],
                        ops[0:64, :], bcs[:])

                # scatter this qt block into its sub's collective input
                s = next(i for i, (sb, qts, _) in enumerate(SUBS)
                         if sb == b and qt in qts)
                sb_, qts, n = SUBS[s]
                nb = 512 // n                     # 128-tok blocks: 4; 64-tok: 8
                c0b = nb * qts.index(qt)
                nc.gpsimd.dma_start(
                    cins[s][c0b:c0b + nb].rearrange("c p t -> p c t"),
                    o_sb[:, tof + qt * 512:tof + qt * 512 + 512]
                    .rearrange("p (c t) -> p c t", c=nb))
                return s

            def emit_a2a(s):
                nc.gpsimd.collective_compute(
                    "AllToAll", mybir.AluOpType.bypass,
                    replica_groups=[list(range(NC))],
                    ins=[cins[s][:].opt()], outs=[couts[s][:].opt()],
                )
                _, _, n = SUBS[s]
                og = SUB_OG[s]
                nc.gpsimd.dma_start(
                    og_all[:, og:og + NC * n]
                    .rearrange("p (c t) -> p c t", c=NC),
                    couts[s][:].rearrange("c p t -> p c t"))

            def emit_outproj(s):
                _, _, n = SUBS[s]
                og = SUB_OG[s]
                row = SUB_ROW[s]
                ob = obpool.tile([128, D], BF16, tag="ob", name="ob")
                for half in range(2):
                    ps = psT.tile([128, 512], F32, tag="sT", name="op")
                    for k in range(NC):
                        nc.tensor.matmul(
                            ps[0:n, :],
                            og_all[:, og + k * n:og + (k + 1) * n],
                            wo_sb[:, k * D + half * 512:
                                  k * D + half * 512 + 512],
                            start=(k == 0), stop=False)
                    nc.tensor.matmul(
                        ps[0:n, :], ones_sb[0:1, 0:n],
                        bo_sb[:, half * 512:half * 512 + 512],
                        start=False, stop=True)
                    nc.scalar.activation(ob[0:n, half * 512:half * 512 + 512],
                                         ps[0:n, :], AF.Identity)
                nc.sync.dma_start(out_ext[row:row + n, :], ob[0:n, :])

            # ---- interleaved schedule
            done_subs = []
            for b in range(B):
                for qt in range(4):
                    tt = b * 4 + qt
                    emit_qkv(tt)
                    s = emit_attention(b, qt)
                    if qt == SUBS[s][1][-1]:
                        emit_a2a(s)
                        done_subs.append(s)
                        if len(done_subs) >= 3:
                            emit_outproj(done_subs[-3])
                    if tt == 2:
                        # deferred so these 2MB don't compete with startup
                        nc.gpsimd.dma_start(wo_sb[:], wo_ext[:])
                        nc.gpsimd.dma_start(bo_sb[:], bo_ext[:])
            emit_outproj(done_subs[-2])
            emit_outproj(done_subs[-1])

            for p in (pso, psT, psA, obpool, nrmpool, atpool, t1pool,
                      xtpool, work, cpool):
                p.release()

    # legalize: split excess sem waits onto preceding same-engine NoOps
    import bass_rust
    from concourse import mybir as _mb
    uid = [0]
    for bb in nc.m.functions[0].blocks:
        il = bb.instructions
        todo = [i for i, inst in enumerate(il)
                if inst.sync_info is not None
                and len(inst.sync_info.on_wait) > 1]
        for idx in reversed(todo):
            inst = il[idx]
            si = inst.sync_info
            waits = list(si.on_wait)
            keep = waits[-1:]
            excess = waits[:-1]
            nops = []
            for i in range(0, len(excess)):
                uid[0] += 1
                nops.append(_mb.InstNoOp(
                    name=f"WSPLIT-{uid[0]}", engine=inst.engine, ins=[], outs=[],
                    bass_nofuse=True,
                    sync_info=bass_rust.SyncInfo(on_wait=excess[i:i + 1],
                                                 on_update=[])))
            inst.sync_info = bass_rust.SyncInfo(on_wait=keep,
                                                on_update=list(si.on_update))
            for j, nop in enumerate(nops):
                il.insert(idx + j, nop)
    return nc


def _wtile(w):
    # [W, D] -> [128, KD*128] with block k = w[:, k*128:(k+1)*128].T
    import ml_dtypes
    BF = ml_dtypes.bfloat16
    kd = w.shape[1] // 128
    return np.ascontiguousarray(
        w.reshape(128, kd, 128).transpose(2, 1, 0).reshape(128, kd * 128)
        .astype(BF))


def _host_prep(x, Wq, bq, Wk, bk, Wv, bv, Wo, bo, scale):
    import ml_dtypes
    BF = ml_dtypes.bfloat16
    s = float(np.asarray(scale).reshape(-1)[0])
    # pre-tiled layout: xT[tt, p, k*512+t] = x[tt*512+t, k*128+p]
    xr = x.reshape(T // 512, 512, D // 128, 128).astype(BF)
    xT = np.ascontiguousarray(xr.transpose(0, 3, 2, 1)
                              .reshape(T // 512, 128, (D // 128) * 512))

    # RoPE tables, feature-major, rows duplicated per interleaved pair
    freqs = THETA ** (-np.arange(0, H, 2, dtype=np.float64) / H)      # [32]
    ang = np.arange(L, dtype=np.float64)[:, None] * freqs[None, :]    # [L, 32]
    cos_t = np.repeat(np.cos(ang).T, 2, axis=0)                       # [64, L]
    sin_t = np.repeat(np.sin(ang).T, 2, axis=0)
    cosT = np.ascontiguousarray(np.tile(cos_t, (HPC, 1)).astype(BF))
    sinT = np.ascontiguousarray(np.tile(sin_t, (HPC, 1)).astype(BF))

    rmat = np.zeros((128, 128), dtype=BF)
    for i in range(64):
        rmat[2 * i + 1, 2 * i] = -1.0
        rmat[2 * i, 2 * i + 1] = 1.0

    ident = np.eye(128, dtype=BF)
    onesc = np.ones((128, 128), dtype=BF)

    # mixed diagonal-block mask: keep key kt for query j when kt <= j
    kt = np.arange(128)[:, None]
    jj = np.arange(128)[None, :]
    mixu = np.where(kt <= jj, 1.0, 0.0).astype(BF)

    woT = np.ascontiguousarray(
        Wo.T.astype(BF).reshape(NC, 128, D).transpose(1, 0, 2)
        .reshape(128, NC * D))
    bo_row = np.ascontiguousarray(bo.astype(BF).reshape(1, D))

    Wq_s = (Wq * s).astype(np.float32)
    bq_s = (bq * s).astype(np.float32)

    in_maps = []
    for c in range(NC):
        hsl = slice(c * W, (c + 1) * W)
        in_maps.append({
            "xT": xT,
            "wq": _wtile(Wq_s[hsl, :]),
            "wk": _wtile(Wk[hsl, :]),
            "wv": _wtile(Wv[hsl, :]),
            "bq": np.ascontiguousarray(bq_s[hsl].reshape(W, 1)),
            "bk": np.ascontiguousarray(bk[hsl].astype(np.float32).reshape(W, 1)),
            "bv": np.ascontiguousarray(bv[hsl].astype(np.float32).reshape(W, 1)),
            "cosT": cosT, "sinT": sinT, "rmat": rmat, "ident": ident,
            "mixu": mixu, "onesc": onesc, "woT": woT, "bo": bo_row,
        })
    return in_maps


def kernel(x, Wq, bq, Wk, bk, Wv, bv, Wo, bo, scale):
    from concourse.bass_utils import run_bass_kernel_spmd

    if "nc" not in _CACHED:
        _CACHED["nc"] = _build()
    nc = _CACHED["nc"]
    in_maps = _host_prep(np.asarray(x), np.asarray(Wq), np.asarray(bq),
                         np.asarray(Wk), np.asarray(bk), np.asarray(Wv),
                         np.asarray(bv), np.asarray(Wo), np.asarray(bo),
                         np.asarray(scale))
    res = run_bass_kernel_spmd(nc, in_maps, list(range(NC)))
    return _assemble(res)


def _assemble(res):
    out = np.empty((T, D), dtype=np.float32)
    base = [0, 1024, 2048, 3072]
    for c in range(NC):
        r = res.results[c]["out"].astype(np.float32)
        for s, (_, _, n) in enumerate(SUBS):
            t0 = base[s] + c * n
            out[t0:t0 + n] = r[SUB_ROW[s]:SUB_ROW[s] + n]
    return out.reshape(B, L, D).astype(np.float32)


# revision 28
# speedup vs baseline: 1.0947x; 1.0947x over previous
"""Causal multi-head attention (RoPE) on 8 TRN2 NeuronCores.

Sharding: Megatron-style head parallelism. Each core owns 2 of the 16 heads:
it computes q/k/v projections for its 128 output features (2 heads x 64),
applies interleaved-pair RoPE (rotation done as a PE matmul with a constant
pair-swap matrix R, tables precomputed host-side), runs causal attention for
its (batch, head) pairs in the transposed orientation S^T = K^T Q so that no
on-chip transposes of the attention matrix are needed, and accumulates
attn^T-weighted V with an extra all-ones V column that yields the softmax
denominators for free. An AllToAll then redistributes the per-head outputs
from head-sharded to token-sharded layout, and each core computes the final
output projection for its token slice. A tiny dummy AllToAll is issued at
kernel start to absorb the one-time collective warmup (~40us) while the
engines compute.

Schedule: qkv(tile t) and attention(qt t) are interleaved per 512-token
block so attention starts as soon as the first tile's k/v are ready, hiding
the input DMA behind compute. Sub-collectives fire per 1024 tokens (the last
1024 split into 2x512 so the tail exposes only a half-size AllToAll).

Causal masking: only the mixed 128x128 block per diagonal chunk is masked
(one small DVE multiply); fully-masked columns are simply skipped by
column-slicing the exp and the AV matmul. Softmax normalization runs off the
PE critical path: reciprocal_approx_fast (DVE) -> partition_broadcast (Pool)
-> one DVE multiply.

Compute dtype: bf16 operands with fp32 PSUM accumulation. (float32r would
be more precise, but f32r matmuls do not register as PE activity for the HAM
clock gate, so the PE stays throttled at 1.2 GHz; bf16 runs at 2.4 GHz.)
"""

import sys

sys.path.insert(0, "/opt/trn_rl_repo")

import numpy as np

B, L, D, N, H = 2, 2048, 1024, 16, 64
T = B * L            # 4096 tokens, batch-major
NC = 8               # cores
HPC = N // NC        # 2 heads per core
W = HPC * H          # 128 projection features per core
THETA = 10000.0
VBLK = 256           # v_sb block stride: [h0 64 | ones 64 | h1 64 | ones 64]

# sub-collectives: (batch, qts, tokens-per-core)
SUBS = [(0, (0, 1), 128), (0, (2, 3), 128), (1, (0, 1), 128),
        (1, (2, 3), 128)]
SUB_ROW = [0, 128, 256, 384]     # out_ext row offset per sub
SUB_OG = [0, 1024, 2048, 3072]   # og_all col offset per sub

_CACHED = {}


def _build():
    import concourse.bass as bass
    import concourse.mybir as mybir
    import concourse.tile as tile

    F32 = mybir.dt.float32
    BF16 = mybir.dt.bfloat16
    AF = mybir.ActivationFunctionType

    # ---- fix: stock _drain_and_barrier overflows the 2-slot sync encoding
    import re as _re
    from concourse.vector_clock import ScopedClock, VectorClock

    def _split_drain_and_barrier(self, tick_clock, wait_clock):
        gc = tick_clock.global_clock
        ticks = [int(v) for v in _re.findall(r"-?\d+", str(gc))]
        for proc, t in enumerate(ticks):
            if t <= 0:
                continue
            sub = VectorClock()
            sub.require_at_least(proc, t)
            d = self.nc.sync.drain()
            wait_clock.add_sem_waits(d.ins, ScopedClock({None: sub}))
        self.nc.all_engine_barrier()
        assert self.sems is not None
        popped = self.nc._tile_sem_poison_stack.pop()
        assert popped is self._sem_poison
        self.nc.clear_and_free_semaphores(list(self.sems.allocated().values()))
        self.nc.all_engine_barrier()

    tile.TileContext._drain_and_barrier = _split_drain_and_barrier

    nc = bass.Bass()

    xT_ext = nc.declare_dram_parameter("xT", [T // 512, 128, (D // 128) * 512], BF16, isOutput=False)
    wq_ext = nc.declare_dram_parameter("wq", [128, D], BF16, isOutput=False)
    wk_ext = nc.declare_dram_parameter("wk", [128, D], BF16, isOutput=False)
    wv_ext = nc.declare_dram_parameter("wv", [128, D], BF16, isOutput=False)
    bq_ext = nc.declare_dram_parameter("bq", [W, 1], F32, isOutput=False)
    bk_ext = nc.declare_dram_parameter("bk", [W, 1], F32, isOutput=False)
    bv_ext = nc.declare_dram_parameter("bv", [W, 1], F32, isOutput=False)
    cos_ext = nc.declare_dram_parameter("cosT", [W, L], BF16, isOutput=False)
    sin_ext = nc.declare_dram_parameter("sinT", [W, L], BF16, isOutput=False)
    rmat_ext = nc.declare_dram_parameter("rmat", [128, 128], BF16, isOutput=False)
    ident_ext = nc.declare_dram_parameter("ident", [128, 128], BF16, isOutput=False)
    mixu_ext = nc.declare_dram_parameter("mixu", [128, 128], BF16, isOutput=False)
    ones_ext = nc.declare_dram_parameter("onesc", [128, 128], BF16, isOutput=False)
    wo_ext = nc.declare_dram_parameter("woT", [128, NC * D], BF16, isOutput=False)
    bo_ext = nc.declare_dram_parameter("bo", [1, D], BF16, isOutput=False)
    out_ext = nc.declare_dram_parameter("out", [T // NC, D], BF16, isOutput=True)

    TT = T // 512      # 8 token tiles of 512
    KD = D // 128      # 8 contraction chunks

    with tile.TileContext(nc) as tc, nc.allow_low_precision(reason="bf16 attn"):
        with tc.tile_pool(name="dram", bufs=1, space="DRAM") as dram:
            # dummy collective: absorbs one-time collective warmup while
            # compute runs
            dum_in = dram.tile([NC, 128, 128], BF16)
            dum_out = dram.tile([NC, 128, 128], BF16)
            nc.gpsimd.collective_compute(
                "AllToAll", mybir.AluOpType.bypass,
                replica_groups=[list(range(NC))],
                ins=[dum_in[:].opt()], outs=[dum_out[:].opt()],
            )
            cins = [dram.tile([NC, 128, n], BF16, name=f"cin{i}")
                    for i, (_, _, n) in enumerate(SUBS)]
            couts = [dram.tile([NC, 128, n], BF16, name=f"cout{i}")
                     for i, (_, _, n) in enumerate(SUBS)]

            cpool = tc.alloc_tile_pool(name="const", bufs=1)
            work = tc.alloc_tile_pool(name="work", bufs=1)
            xtpool = tc.alloc_tile_pool(name="xt", bufs=3)
            t1pool = tc.alloc_tile_pool(name="p1t", bufs=3)
            atpool = tc.alloc_tile_pool(name="att", bufs=6)
            nrmpool = tc.alloc_tile_pool(name="nrm", bufs=2)
            obpool = tc.alloc_tile_pool(name="ob", bufs=2)
            psA = tc.alloc_tile_pool(name="psA", bufs=2, space="PSUM")
            psT = tc.alloc_tile_pool(name="psT", bufs=2, space="PSUM")
            pso = tc.alloc_tile_pool(name="pso", bufs=2, space="PSUM")

            # ---- constants (spread across DMA queues for startup bandwidth)
            wq_sb = cpool.tile([128, KD * 128], BF16)
            nc.scalar.dma_start(wq_sb[:], wq_ext[:])
            bq_sb = cpool.tile([W, 1], F32)
            nc.scalar.dma_start(bq_sb[:], bq_ext[:])
            wk_sb = cpool.tile([128, KD * 128], BF16)
            nc.scalar.dma_start(wk_sb[:], wk_ext[:])
            wv_sb = cpool.tile([128, KD * 128], BF16)
            nc.scalar.dma_start(wv_sb[:], wv_ext[:])
            bk_sb = cpool.tile([W, 1], F32)
            nc.scalar.dma_start(bk_sb[:], bk_ext[:])
            bv_sb = cpool.tile([W, 1], F32)
            nc.scalar.dma_start(bv_sb[:], bv_ext[:])
            rmat_sb = cpool.tile([128, 128], BF16)
            nc.scalar.dma_start(rmat_sb[:], rmat_ext[:])
            ident_sb = cpool.tile([128, 128], BF16)
            nc.scalar.dma_start(ident_sb[:], ident_ext[:])
            mixu_sb = cpool.tile([128, 128], BF16)
            nc.scalar.dma_start(mixu_sb[:], mixu_ext[:])
            # cos/sin loaded per-512-column slice inside emit_qkv so they
            # never delay the first x tile
            cos_sb = cpool.tile([W, L], BF16)
            sin_sb = cpool.tile([W, L], BF16)
            ones_sb = cpool.tile([128, 128], BF16)
            nc.gpsimd.dma_start(ones_sb[:], ones_ext[:])
            wo_sb = cpool.tile([128, NC * D], BF16)
            bo_sb = cpool.tile([1, D], BF16)

            # ---- long-lived working tensors
            q_ro = work.tile([128, T], BF16)
            k_ro = work.tile([128, T], BF16)
            v_sb = work.tile([128, 32 * VBLK], BF16)
            o_sb = work.tile([128, T], BF16)
            og_all = work.tile([128, T], BF16)
            # ones blocks of v_sb: the AV stationary is [v(64) | ones(64)]
            # per head so PSUM rows 64-127 come out holding 64 copies of the
            # softmax denominator (no separate broadcast needed)
            nc.vector.memset(v_sb[:], 1.0)

            def emit_qkv(tt):
                xt = xtpool.tile([128, KD * 512], BF16, name="xt", tag="xt")
                if tt == 0:
                    for quarter in range(4):
                        c0 = quarter * KD * 128
                        nc.sync.dma_start(xt[:, c0:c0 + KD * 128],
                                          xT_ext[tt][:, c0:c0 + KD * 128])
                elif tt % 2 == 0:
                    nc.sync.dma_start(xt[:], xT_ext[tt])
                else:
                    nc.gpsimd.dma_start(xt[:], xT_ext[tt])
                xts = [xt[:, k * 512:(k + 1) * 512] for k in range(KD)]
                lcol = (tt % (TT // B)) * 512
                if tt < TT // B:
                    nc.sync.dma_start(cos_sb[:, lcol:lcol + 512],
                                      cos_ext[:, lcol:lcol + 512])
                    nc.sync.dma_start(sin_sb[:, lcol:lcol + 512],
                                      sin_ext[:, lcol:lcol + 512])

                for wsb, bsb, dst in ((wq_sb, bq_sb, q_ro),
                                      (wk_sb, bk_sb, k_ro)):
                    ps = psA.tile([128, 512], F32, tag="pj", name="pj")
                    for k in range(KD):
                        nc.tensor.matmul(ps[:], wsb[:, k * 128:(k + 1) * 128],
                                         xts[k], start=(k == 0),
                                         stop=(k == KD - 1))
                    bs = t1pool.tile([128, 512], BF16, tag="bs", name="bs")
                    nc.scalar.activation(bs[:], ps[:], AF.Identity, bias=bsb[:])
                    rot = psA.tile([128, 512], F32, tag="pj", name="rot")
                    nc.tensor.matmul(rot[:], rmat_sb[:], bs[:],
                                     start=True, stop=True)
                    t1 = t1pool.tile([128, 512], BF16, tag="t1", name="t1")
                    nc.vector.tensor_mul(t1[:], bs[:], cos_sb[:, lcol:lcol + 512])
                    t2 = t1pool.tile([128, 512], BF16, tag="t2", name="t2")
                    nc.vector.tensor_mul(t2[:], rot[:], sin_sb[:, lcol:lcol + 512])
                    nc.vector.tensor_add(dst[:, tt * 512:(tt + 1) * 512],
                                         t1[:], t2[:])

                ps = psA.tile([128, 512], F32, tag="pj", name="pv")
                for k in range(KD):
                    nc.tensor.matmul(ps[:], wv_sb[:, k * 128:(k + 1) * 128],
                                     xts[k], start=(k == 0), stop=(k == KD - 1))
                vbs = t1pool.tile([128, 512], BF16, tag="bs", name="vbs")
                nc.scalar.activation(vbs[:], ps[:], AF.Identity, bias=bv_sb[:])
                for s in range(4):
                    vt = psA.tile([128, 128], BF16, tag="pj", name="vt")
                    nc.tensor.transpose(vt[:], vbs[:, s * 128:(s + 1) * 128],
                                        ident_sb[:])
                    blk = (tt * 4 + s) * VBLK
                    nc.vector.tensor_copy(v_sb[:, blk:blk + 64], vt[:, 0:64])
                    nc.vector.tensor_copy(v_sb[:, blk + 128:blk + 192],
                                          vt[:, 64:128])

            def emit_attention(b, qt):
                tof = b * L
                vb = b * (L // 128)
                nkc = 4 * qt + 4
                opss = [pso.tile([128, 512], F32, tag="ops", name="ops")
                        for _ in range(HPC)]
                ats = {}

                def emit_sT(kc):
                    r = kc - 4 * qt
                    cs = 128 * r if r > 0 else 0    # skip fully-masked cols
                    sT2 = psT.tile([128, 1024], F32, tag="sT", name="sT")
                    for hl in range(HPC):
                        nc.tensor.matmul(
                            sT2[:, hl * 512 + cs:(hl + 1) * 512],
                            k_ro[64 * hl:64 * hl + 64,
                                 tof + kc * 128:tof + kc * 128 + 128],
                            q_ro[64 * hl:64 * hl + 64,
                                 tof + qt * 512 + cs:tof + qt * 512 + 512],
                            start=True, stop=True)
                    at2 = atpool.tile([128, 1024], BF16, tag="at", name="at")
                    if r <= 0:
                        nc.scalar.activation(at2[:], sT2[:], AF.Exp)
                    else:
                        c0 = 128 * r
                        for hl in range(HPC):
                            nc.scalar.activation(
                                at2[:, hl * 512 + c0:(hl + 1) * 512],
                                sT2[:, hl * 512 + c0:(hl + 1) * 512], AF.Exp)
                    if r >= 0:
                        c0 = 128 * r
                        for hl in range(HPC):
                            nc.vector.tensor_mul(
                                at2[:, hl * 512 + c0:hl * 512 + c0 + 128],
                                at2[:, hl * 512 + c0:hl * 512 + c0 + 128],
                                mixu_sb[:])
                    ats[kc] = at2

                def emit_av(kc, first, last):
                    r = kc - 4 * qt
                    c0 = 128 * r if r > 0 else 0
                    for hl in range(HPC):
                        nc.tensor.matmul(
                            opss[hl][:, c0:512],
                            v_sb[:, (vb + kc) * VBLK + 128 * hl:
                                 (vb + kc) * VBLK + 128 * hl + 128],
                            ats[kc][:, hl * 512 + c0:(hl + 1) * 512],
                            start=(kc == first), stop=(kc == last),
                            skip_group_check=True)

                # diagonal (masked) chunks first
                kcs = list(range(4 * qt, nkc)) + list(range(0, 4 * qt))
                first, last = kcs[0], kcs[-1]
                emit_sT(first)
                for i in range(1, nkc):
                    emit_sT(kcs[i])
                    emit_av(kcs[i - 1], first, last)
                emit_av(last, first, last)

                for hl in range(HPC):
                    hof = 64 * hl
                    ops = opss[hl]
                    # 1/d = exp(-ln d); rows 64-127 hold 64 denominator copies
                    lnt = nrmpool.tile([64, 512], F32, tag="lnt", name="lnt")
                    nc.scalar.activation(lnt[:], ops[64:128, :], AF.Ln)
                    bcs = nrmpool.tile([64, 512], F32, tag="bcs", name="bcs")
                    nc.scalar.activation(bcs[:], lnt[:], AF.Exp, scale=-1.0)
                    nc.vector.tensor_mul(
                        o_sb[hof:hof + 64,
                             tof + qt * 512:tof + qt * 512 + 512],
                        ops[0:64, :], bcs[:])

                # scatter this qt block into its sub's collective input
                s = next(i for i, (sb, qts, _) in enumerate(SUBS)
                         if sb == b and qt in qts)
                sb_, qts, n = SUBS[s]
                nb = 512 // n                     # 128-tok blocks: 4; 64-tok: 8
                c0b = nb * qts.index(qt)
                nc.gpsimd.dma_start(
                    cins[s][c0b:c0b + nb].rearrange("c p t -> p c t"),
                    o_sb[:, tof + qt * 512:tof + qt * 512 + 512]
                    .rearrange("p (c t) -> p c t", c=nb))
                return s

            def emit_a2a(s):
                nc.gpsimd.collective_compute(
                    "AllToAll", mybir.AluOpType.bypass,
                    replica_groups=[list(range(NC))],
                    ins=[cins[s][:].opt()], outs=[couts[s][:].opt()],
                )
                _, _, n = SUBS[s]
                og = SUB_OG[s]
                nc.gpsimd.dma_start(
                    og_all[:, og:og + NC * n]
                    .rearrange("p (c t) -> p c t", c=NC),
                    couts[s][:].rearrange("c p t -> p c t"))

            def emit_outproj(s):
                _, _, n = SUBS[s]
                og = SUB_OG[s]
                row = SUB_ROW[s]
                ob = obpool.tile([128, D], BF16, tag="ob", name="ob")
                for half in range(2):
                    ps = psT.tile([128, 512], F32, tag="sT", name="op")
                    for k in range(NC):
                        nc.tensor.matmul(
                            ps[0:n, :],
                            og_all[:, og + k * n:og + (k + 1) * n],
                            wo_sb[:, k * D + half * 512:
                                  k * D + half * 512 + 512],
                            start=(k == 0), stop=False)
                    nc.tensor.matmul(
                        ps[0:n, :], ones_sb[0:1, 0:n],
                        bo_sb[:, half * 512:half * 512 + 512],
                        start=False, stop=True)
                    nc.scalar.activation(ob[0:n, half * 512:half * 512 + 512],
                                         ps[0:n, :], AF.Identity)
                nc.sync.dma_start(out_ext[row:row + n, :], ob[0:n, :])

            # ---- interleaved schedule
            done_subs = []
            for b in range(B):
                for qt in range(4):
                    tt = b * 4 + qt
                    emit_qkv(tt)
                    s = emit_attention(b, qt)
                    if qt == SUBS[s][1][-1]:
                        emit_a2a(s)
                        done_subs.append(s)
                        if len(done_subs) >= 3:
                            emit_outproj(done_subs[-3])
                    if tt == 2:
                        # deferred so these 2MB don't compete with startup
                        nc.gpsimd.dma_start(wo_sb[:], wo_ext[:])
                        nc.gpsimd.dma_start(bo_sb[:], bo_ext[:])
            emit_outproj(done_subs[-2])
            emit_outproj(done_subs[-1])

            for p in (pso, psT, psA, obpool, nrmpool, atpool, t1pool,
                      xtpool, work, cpool):
                p.release()

    # legalize: split excess sem waits onto preceding same-engine NoOps
    import bass_rust
    from concourse import mybir as _mb
    uid = [0]
    for bb in nc.m.functions[0].blocks:
        il = bb.instructions
        todo = [i for i, inst in enumerate(il)
                if inst.sync_info is not None
                and len(inst.sync_info.on_wait) > 1]
        for idx in reversed(todo):
            inst = il[idx]
            si = inst.sync_info
            waits = list(si.on_wait)
            keep = waits[-1:]
            excess = waits[:-1]
            nops = []
            for i in range(0, len(excess)):
                uid[0] += 1
                nops.append(_mb.InstNoOp(
                    name=f"WSPLIT-{uid[0]}", engine=inst.engine, ins=[], outs=[],
                    bass_nofuse=True,
                    sync_info=bass_rust.SyncInfo(on_wait=excess[i:i + 1],
                                                 on_update=[])))
            inst.sync_info = bass_rust.SyncInfo(on_wait=keep,
                                                on_update=list(si.on_update))
            for j, nop in enumerate(nops):
                il.insert(idx + j, nop)
    return nc


def _wtile(w):
    # [W, D] -> [128, KD*128] with block k = w[:, k*128:(k+1)*128].T
    import ml_dtypes
    BF = ml_dtypes.bfloat16
    kd = w.shape[1] // 128
    return np.ascontiguousarray(
        w.reshape(128, kd, 128).transpose(2, 1, 0).reshape(128, kd * 128)
        .astype(BF))


def _host_prep(x, Wq, bq, Wk, bk, Wv, bv, Wo, bo, scale):
    import ml_dtypes
    BF = ml_dtypes.bfloat16
    s = float(np.asarray(scale).reshape(-1)[0])
    # pre-tiled layout: xT[tt, p, k*512+t] = x[tt*512+t, k*128+p]
    xr = x.reshape(T // 512, 512, D // 128, 128).astype(BF)
    xT = np.ascontiguousarray(xr.transpose(0, 3, 2, 1)
                              .reshape(T // 512, 128, (D // 128) * 512))

    # RoPE tables, feature-major, rows duplicated per interleaved pair
    freqs = THETA ** (-np.arange(0, H, 2, dtype=np.float64) / H)      # [32]
    ang = np.arange(L, dtype=np.float64)[:, None] * freqs[None, :]    # [L, 32]
    cos_t = np.repeat(np.cos(ang).T, 2, axis=0)                       # [64, L]
    sin_t = np.repeat(np.sin(ang).T, 2, axis=0)
    cosT = np.ascontiguousarray(np.tile(cos_t, (HPC, 1)).astype(BF))
    sinT = np.ascontiguousarray(np.tile(sin_t, (HPC, 1)).astype(BF))

    rmat = np.zeros((128, 128), dtype=BF)
    for i in range(64):
        rmat[2 * i + 1, 2 * i] = -1.0
        rmat[2 * i, 2 * i + 1] = 1.0

    ident = np.eye(128, dtype=BF)
    onesc = np.ones((128, 128), dtype=BF)

    # mixed diagonal-block mask: keep key kt for query j when kt <= j
    kt = np.arange(128)[:, None]
    jj = np.arange(128)[None, :]
    mixu = np.where(kt <= jj, 1.0, 0.0).astype(BF)

    woT = np.ascontiguousarray(
        Wo.T.astype(BF).reshape(NC, 128, D).transpose(1, 0, 2)
        .reshape(128, NC * D))
    bo_row = np.ascontiguousarray(bo.astype(BF).reshape(1, D))

    Wq_s = (Wq * s).astype(np.float32)
    bq_s = (bq * s).astype(np.float32)

    in_maps = []
    for c in range(NC):
        hsl = slice(c * W, (c + 1) * W)
        in_maps.append({
            "xT": xT,
            "wq": _wtile(Wq_s[hsl, :]),
            "wk": _wtile(Wk[hsl, :]),
            "wv": _wtile(Wv[hsl, :]),
            "bq": np.ascontiguousarray(bq_s[hsl].reshape(W, 1)),
            "bk": np.ascontiguousarray(bk[hsl].astype(np.float32).reshape(W, 1)),
            "bv": np.ascontiguousarray(bv[hsl].astype(np.float32).reshape(W, 1)),
            "cosT": cosT, "sinT": sinT, "rmat": rmat, "ident": ident,
            "mixu": mixu, "onesc": onesc, "woT": woT, "bo": bo_row,
        })
    return in_maps


def kernel(x, Wq, bq, Wk, bk, Wv, bv, Wo, bo, scale):
    from concourse.bass_utils import run_bass_kernel_spmd

    if "nc" not in _CACHED:
        _CACHED["nc"] = _build()
    nc = _CACHED["nc"]
    in_maps = _host_prep(np.asarray(x), np.asarray(Wq), np.asarray(bq),
                         np.asarray(Wk), np.asarray(bk), np.asarray(Wv),
                         np.asarray(bv), np.asarray(Wo), np.asarray(bo),
                         np.asarray(scale))
    res = run_bass_kernel_spmd(nc, in_maps, list(range(NC)))
    return _assemble(res)


def _assemble(res):
    out = np.empty((T, D), dtype=np.float32)
    base = [0, 1024, 2048, 3072]
    for c in range(NC):
        r = res.results[c]["out"].astype(np.float32)
        for s, (_, _, n) in enumerate(SUBS):
            t0 = base[s] + c * n
            out[t0:t0 + n] = r[SUB_ROW[s]:SUB_ROW[s] + n]
    return out.reshape(B, L, D).astype(np.float32)
